# revision 14
# baseline (speedup 1.0000x reference)
import sys

sys.path.insert(0, "/opt/trn_rl_repo")

import numpy as np
import ml_dtypes

import concourse.bass as bass
from concourse import bacc
import concourse.mybir as mybir
from concourse.tile import TileContext
from concourse import bass_utils

_SPARSE = (48, 64, 96, 128, 192, 256, 384, 512, 768, 1024, 1536)
OFFSETS = tuple(range(33)) + _SPARSE
B, H, N, HD = 4, 16, 4096, 64
BDS = [0, 1, 2, 3, 4, 6, 8, 12]  # block distances (x128) covering all offsets
NEAR = [0, 1, 2]
FAR = [3, 4, 6, 8, 12]
SW = 3 * 128 + 5 * 64  # 704 score cols: 3 near blocks + 5 far 64-wide half-stacked
NBD = len(BDS)
NT = N // 128  # 32 q tiles per (b,h)
NBH = 8  # bh pairs per core
NCORES = 8
VW = 66  # v columns: 64 hd + 1 ones + 1 pad
BF16 = ml_dtypes.bfloat16

_nc_cache = None


def _build_bass():
    nc = bacc.Bacc("TRN2", target_bir_lowering=False)
    qkt = nc.dram_tensor("qkt", [NBH, 64, 2 * N], mybir.dt.bfloat16, kind="ExternalInput")
    va = nc.dram_tensor("va", [NBH, 128, NT * VW], mybir.dt.bfloat16, kind="ExternalInput")
    em = nc.dram_tensor("em", [NBH, 128, SW], mybir.dt.bfloat16, kind="ExternalInput")
    out = nc.dram_tensor("out", [NBH, N, 64], mybir.dt.float32, kind="ExternalOutput")

    with TileContext(nc) as tc:
        with (
            tc.tile_pool(name="io", bufs=3) as io_pool,
            tc.tile_pool(name="spsum", bufs=3, space="PSUM") as spool,
            tc.tile_pool(name="opsum", bufs=2, space="PSUM") as opool,
            tc.tile_pool(name="work", bufs=4) as wpool,
        ):
            for bh in range(NBH):
                qk_sb = io_pool.tile([64, 2 * N], mybir.dt.bfloat16, tag="qkt")
                qt_sb = qk_sb[:, :N]
                kt_sb = qk_sb[:, N:]
                va_sb = io_pool.tile([128, NT * VW], mybir.dt.bfloat16, tag="va")
                em_sb = io_pool.tile([128, SW], mybir.dt.bfloat16, tag="em")
                nc.sync.dma_start(qk_sb[:, :], qkt[bh])
                nc.sync.dma_start(va_sb[:, :], va[bh])
                nc.sync.dma_start(em_sb[:, :], em[bh])
                em_c = io_pool.tile([128, SW], mybir.dt.bfloat16, tag="em_c")
                nc.vector.tensor_copy(em_c[:, :], em_sb[:, :])
                obig = io_pool.tile([128, NT * 64], mybir.dt.float32, tag="obig")
                for t in range(NT):
                    nn = sum(1 for bd in NEAR if t - bd >= 0)
                    nf = sum(1 for bd in FAR if t - bd >= 0)
                    w = nn * 128 + nf * 64
                    sp = spool.tile([128, SW], mybir.dt.float32, tag="sp")
                    for i in range(nn):
                        bd = NEAR[i]
                        nc.tensor.matmul(
                            sp[:, i * 128 : (i + 1) * 128],
                            kt_sb[:, (t - bd) * 128 : (t - bd + 1) * 128],
                            qt_sb[:, t * 128 : (t + 1) * 128],
                            start=True,
                            stop=True,
                        )
                    for f in range(nf):
                        bd = FAR[f]
                        c0 = 384 + f * 64
                        for h in (0, 1):
                            nc.tensor.matmul(
                                sp[64 * h : 64 * h + 64, c0 : c0 + 64],
                                kt_sb[:, (t - bd) * 128 + 64 * h : (t - bd) * 128 + 64 * h + 64],
                                qt_sb[:, t * 128 + 64 * h : t * 128 + 64 * h + 64],
                                start=True,
                                stop=True,
                                tile_position=(0, 64 * h),
                            )
                    ex = wpool.tile([128, SW], mybir.dt.bfloat16, tag="ex")
                    nc.scalar.activation(
                        ex[:, :w], sp[:, :w],
                        mybir.ActivationFunctionType.Exp,
                    )
                    exm = wpool.tile([128, SW], mybir.dt.bfloat16, tag="exm")
                    nc.vector.tensor_mul(
                        exm[:, :w], ex[:, :w], em_c[:, :w]
                    )
                    op = opool.tile([128, VW], mybir.dt.float32, tag="op")
                    for i in range(nn):
                        bd = NEAR[i]
                        nc.tensor.matmul(
                            op[:, :],
                            exm[:, i * 128 : (i + 1) * 128],
                            va_sb[:, (t - bd) * VW : (t - bd + 1) * VW],
                            start=(i == 0),
                            stop=(i == nn - 1 and nf == 0),
                        )
                    for f in range(nf):
                        bd = FAR[f]
                        c0 = 384 + f * 64
                        for h in (0, 1):
                            nc.tensor.matmul(
                                op[64 * h : 64 * h + 64, :],
                                exm[64 * h : 64 * h + 64, c0 : c0 + 64],
                                va_sb[64 * h : 64 * h + 64, (t - bd) * VW : (t - bd + 1) * VW],
                                start=False,
                                stop=(f == nf - 1 and h == 1),
                                tile_position=(64 * h, 64 * h),
                                skip_group_check=True,
                            )
                    rec = wpool.tile([128, 1], mybir.dt.float32, tag="rec")
                    nc.vector.reciprocal(rec[:, :], op[:, 64:65])
                    nc.vector.tensor_scalar_mul(
                        obig[:, t * 64 : (t + 1) * 64], op[:, :64], rec[:, :]
                    )
                out_v = out[bh].rearrange("(t p) c -> p t c", p=128)
                nc.sync.dma_start(out_v, obig[:, :].rearrange("p (t c) -> p t c", c=64))
    nc.compile()
    return nc


def _host_prep(q, k, v, pos_bias):
    # flatten (b,h) -> 64 pairs; core i gets [8i, 8i+8)
    qf = q.reshape(B * H, N, HD)
    kf = k.reshape(B * H, N, HD)
    vf = v.reshape(B * H, N, HD)
    sc = 1.0 / np.sqrt(HD)

    # per-head exp-mask [128, NBD*128]
    lut = np.full(1537, -1, np.int64)
    for i, d in enumerate(OFFSETS):
        lut[d] = i
    m_i = np.arange(128)[:, None]
    n_i = np.arange(128)[None, :]
    em_heads = np.zeros((H, 128, SW), np.float32)
    for bdi, bd in enumerate(NEAR):
        dd = 128 * bd + n_i - m_i  # [128,128]
        ok = (dd >= 0) & (dd <= 1536)
        idx = np.where(ok, lut[np.clip(dd, 0, 1536)], -1)
        valid = idx >= 0
        for h in range(H):
            vals = np.where(valid, np.exp(pos_bias[np.clip(idx, 0, 43), h]), 0.0)
            em_heads[h, :, bdi * 128 : (bdi + 1) * 128] = vals
    for f, bd in enumerate(FAR):
        i_off = lut[128 * bd]
        diag = (np.arange(128)[:, None] % 64) == np.arange(64)[None, :]
        for h in range(H):
            em_heads[h, :, 384 + f * 64 : 384 + (f + 1) * 64] = diag * np.exp(pos_bias[i_off, h])

    in_maps = []
    for c in range(NCORES):
        bhs = range(c * NBH, (c + 1) * NBH)
        qkt = np.concatenate(
            [np.stack([(qf[j].T * sc) for j in bhs]),
             np.stack([kf[j].T for j in bhs])], axis=2).astype(BF16)
        va = np.zeros((NBH, 128, NT * VW), np.float32)
        for jj, j in enumerate(bhs):
            vt = vf[j].reshape(NT, 128, HD)  # [t, p, hd]
            va[jj, :, :] = np.concatenate(
                [vt, np.ones((NT, 128, 1), np.float32), np.zeros((NT, 128, 1), np.float32)],
                axis=2,
            ).transpose(1, 0, 2).reshape(128, NT * VW)
        em = np.stack([em_heads[j % H] for j in bhs]).astype(BF16)
        in_maps.append({
            "qkt": np.ascontiguousarray(qkt),
            "va": np.ascontiguousarray(va.astype(BF16)),
            "em": np.ascontiguousarray(em),
        })
    return in_maps


def kernel(q, k, v, pos_bias):
    global _nc_cache
    if _nc_cache is None:
        _nc_cache = _build_bass()
    nc = _nc_cache
    in_maps = _host_prep(
        np.asarray(q, np.float32), np.asarray(k, np.float32),
        np.asarray(v, np.float32), np.asarray(pos_bias, np.float32),
    )
    res = bass_utils.run_bass_kernel_spmd(nc, in_maps, core_ids=list(range(NCORES)))
    outs = [r["out"] for r in res.results]
    full = np.concatenate(outs, axis=0).reshape(B, H, N, HD)
    return full.astype(np.float32)
